# revision 24
# baseline (speedup 1.0000x reference)
"""NVFP4 block-quantized linear layer (x @ w.T + bias) on 8 Trainium2 cores.

Reference semantics (reference.py): both activations and weights are
block-quantized along K (blocks of 16) to fp4-e2m1 with e4m3 scales
(scale = absmax/6, round-to-nearest), dequantized, then matmul with fp32
accumulation, cast to bf16, plus bf16 bias.

Device strategy: pure column-parallel over 8 cores (N 12288 -> 1536/core),
x replicated.  Per core:
  - weights quantized once (12 row-tiles), xbar-DMA-transposed into a
    resident SBUF tensor WT [128k, 12nr, 24kc, 128n] bf16 (74KB/partition).
  - activations quantized per 128-row tile on VectorE (blockwise absmax,
    e4m3 RNE scale via per-block magic-add, fp4 round via two custom DVE
    ops), xbar-DMA-transposed to XT [128k, 24kc, 128m].
  - TensorE: for each m-tile, 24kc x 3nb bf16 matmuls accumulate in 6
    PSUM banks (double-buffered across m-tiles); ScalarE evacuates to
    bf16, GpSimd adds bias (bf16, matching the reference's double
    rounding), one 384KB DMA stores each m-tile row block.
All DMA is HWDGE; transposes ride the DMA xbar so TensorE runs nothing
but the 4608 matmuls (216ns each at N=512 -> ~1ms roofline).
"""

import numpy as np
import ml_dtypes

f32 = np.float32
bf16 = ml_dtypes.bfloat16

# ---------------------------------------------------------------------------
# problem geometry (hardcoded; harness calls kernel() with these full shapes)
B, T, K = 2, 4096, 3072
N = 12288
M = B * T                      # 8192
GRID_M, GRID_N = 1, 8          # column-parallel
M_CORE = M                     # 8192
N_CORE = N // GRID_N           # 1536
NUM_CORES = GRID_M * GRID_N

KC = K // 128                  # 24 k-chunks
KB = K // 16                   # 192 scale blocks per row
NR = N_CORE // 128             # 12 weight row-tiles
MT = M_CORE // 128             # 64 x row-tiles
NB = N_CORE // 512             # 3 psum n-blocks
TPB = 4                        # w row-tiles per 512-wide n-block

CH1 = float(1.5 * 2**22)
RCP6 = float(f32(1.0) / f32(6.0))
MAGIC = float(1.5 * 2**20)     # pe2*MAGIC has ulp = pe2/8 (e4m3 mantissa grid)

_BUILT = None


# ---------------------------------------------------------------------------
def _register_custom_ops():
    """Register the two fp4-rounding custom DVE ops (idempotent)."""
    import concourse.dve_ops as dve_ops
    from concourse.dve_ops import DveOp, OPS, _SUB_OPCODE_FOR_NAME, _CUSTOM_DVE_ROW_BASE
    from concourse.dve_spec import (
        Spec, Src0, Src1, C0, C1, Zero, One, AluOp, Bin,
        maxx, minn, select, lower, _has_src1,
    )
    from concourse.dve_uop import DveOpSpec

    def _norm2(in0, in1):
        in0 = np.asarray(in0)
        in1 = np.asarray(in1)
        if in1.size != in0.size:
            in1 = np.broadcast_to(in1, in0.shape)
        return in0, np.ascontiguousarray(in1).reshape(in0.shape)

    def _ref_fp4_pre(in0, in1, s0, s1, imm2=None):
        in0, in1 = _norm2(in0, in1)
        m = (in0.astype(f32) * in1.astype(f32)).astype(f32)
        s2 = (m * m).astype(f32)
        ch = np.where(
            s2 < f32(4.0), f32(CH1),
            ((f32(1.0) + (s2 >= f32(16.0)).astype(f32)) * f32(1.5 * 2**23)).astype(f32),
        ).astype(f32)
        return (m + ch).astype(f32)

    def _ref_fp4_fin(in0, in1, s0, s1, imm2=None):
        in0, in1 = _norm2(in0, in1)
        qpre = np.ascontiguousarray(in0.astype(f32))
        pe = (qpre.view(np.uint32) & np.uint32(0x7F800000)).view(f32)
        d1 = (qpre - pe).astype(f32)
        q2 = ((d1 + d1).astype(f32) - pe).astype(f32)
        qc = np.maximum(np.minimum(q2, f32(12.0)), f32(-12.0))
        return (qc * in1.astype(f32)).astype(f32)

    def build_pre():
        SIXTEEN = C0 * C0
        Ch2x = C1 + C1
        m = Src0 * Src1
        s2 = m * m
        c2 = s2 >= SIXTEEN
        inner = (c2 + One) * Ch2x
        c1 = s2 < C0
        outer = select(c1, C1, inner)
        return Spec(body=m + outer, reference=_ref_fp4_pre)

    def build_fin():
        pe = Bin(AluOp.BITWISE_AND, Src0, C0)
        d1 = Src0 - pe
        q2 = (d1 + d1) - pe
        qc = maxx(minn(q2, C1), Zero - C1)
        return Spec(body=qc * Src1, reference=_ref_fp4_fin)

    def register(name, spec):
        if name in _SUB_OPCODE_FOR_NAME:
            for op in OPS:
                if op.name == name:
                    return op
            raise RuntimeError(name)
        row = _CUSTOM_DVE_ROW_BASE + len(OPS)
        assert row < 0x20
        shas = {}
        for ver in ("v3", "v4"):
            try:
                uops = lower(spec, ver=ver)
            except Exception:
                continue
            shas[ver] = DveOpSpec(
                name=name, opcode=row, uops=uops, rd1_en=_has_src1(spec)
            ).sha(ver)
        op = DveOp(name, spec, subdim=False, uops_sha=shas)
        OPS.append(op)
        _SUB_OPCODE_FOR_NAME[name] = row
        dve_ops.CUSTOM_DVE_SPECS[name] = spec
        return op

    return register("FP4_PRE_ANT", build_pre()), register("FP4_FIN_ANT", build_fin())


# ---------------------------------------------------------------------------
def _patch_tile_drain():
    """The TileContext tail drain attaches one sem-wait per live logical
    processor to a single SP Drain instruction; this walrus build caps sync
    waits per instruction at 2 ("Too many sync wait commands").  Split the
    overflow waits onto preceding single-wait SP nops (sound: all waits still
    complete before the post-drain all-engine barrier / sem reset)."""
    from concourse import tile as tile_mod
    import concourse.mybir as mybir
    from concourse.vector_clock import ScopedClock

    if getattr(tile_mod.TileContext, "_ant_drain_patched", False):
        return

    def _drain_and_barrier(self, tick_clock, wait_clock):
        nc = self.nc
        probe = nc.sync.nop()
        wait_clock.add_sem_waits(
            probe.ins, ScopedClock({None: tick_clock.global_clock})
        )
        si = probe.ins.sync_info
        waits = list(si.on_wait) if si is not None and si.on_wait else []
        if len(waits) > 1:
            probe.ins.sync_info = mybir.SyncInfo(
                on_wait=waits[:1],
                on_update=list(si.on_update) if si.on_update else [],
            )
            for w in waits[1:]:
                extra = nc.sync.nop()
                extra.ins.sync_info = mybir.SyncInfo(on_wait=[w], on_update=[])
        nc.sync.drain()

        nc.all_engine_barrier()
        assert self.sems is not None
        popped = nc._tile_sem_poison_stack.pop()
        assert popped is self._sem_poison
        nc.clear_and_free_semaphores(list(self.sems.allocated().values()))
        nc.all_engine_barrier()

    tile_mod.TileContext._drain_and_barrier = _drain_and_barrier
    tile_mod.TileContext._ant_drain_patched = True


def _split_excess_waits(nc, max_waits=1):
    """This walrus build rejects instructions carrying more than `max_waits`
    sem waits ("Too many sync wait commands").  Move overflow waits onto
    same-engine NoOp instructions inserted immediately before the offender —
    per-engine program order makes this semantically identical."""
    import concourse.mybir as mybir

    ctr = [0]
    for f in nc.m.functions:
        for blk in f.blocks:
            il = blk.instructions
            out = []
            changed = False
            for ins in il:
                si = ins.sync_info
                waits = list(si.on_wait) if si is not None and si.on_wait else []
                if len(waits) > max_waits:
                    changed = True
                    extra = waits[:-max_waits]
                    for i0 in range(0, len(extra), max_waits):
                        nop = mybir.InstNoOp(
                            name=f"I-waitsplit-{ctr[0]}", ins=[], outs=[])
                        ctr[0] += 1
                        nop.engine = ins.engine
                        nop.sync_info = mybir.SyncInfo(
                            on_wait=extra[i0:i0 + max_waits], on_update=[])
                        out.append(nop)
                    ins.sync_info = mybir.SyncInfo(
                        on_wait=waits[-max_waits:],
                        on_update=list(si.on_update) if si.on_update else [],
                    )
                out.append(ins)
            if changed:
                blk.instructions = out


def build_nc(num_cores=NUM_CORES, debug=False, postprocess=True):
    """Build the per-core Bass program (SPMD: same program on every core)."""
    import concourse.bass as bass
    import concourse.mybir as mybir
    from concourse import tile
    from contextlib import ExitStack

    fp4_pre, fp4_fin = _register_custom_ops()
    _patch_tile_drain()

    nc = bass.Bass("TRN2", target_bir_lowering=False, debug=debug,
                   num_devices=num_cores)
    dt = mybir.dt
    Alu = mybir.AluOpType

    x_d = nc.dram_tensor("x", [M_CORE, K], dt.float32, kind="ExternalInput")
    w_d = nc.dram_tensor("w", [N_CORE, K], dt.float32, kind="ExternalInput")
    b_d = nc.dram_tensor("bias", [N_CORE], dt.bfloat16, kind="ExternalInput")
    out_d = nc.dram_tensor("out", [M_CORE, N_CORE], dt.bfloat16,
                           kind="ExternalOutput")

    with tile.TileContext(nc) as tc, ExitStack() as ctx:
        cst = ctx.enter_context(tc.tile_pool(name="cst", bufs=1))
        wtp = ctx.enter_context(tc.tile_pool(name="wtp", bufs=1))
        xtp = ctx.enter_context(tc.tile_pool(name="xtp", bufs=4))
        stg = ctx.enter_context(tc.tile_pool(name="stg", bufs=2))
        qpp = ctx.enter_context(tc.tile_pool(name="qpp", bufs=1))
        dqp = ctx.enter_context(tc.tile_pool(name="dqp", bufs=2))
        scp = ctx.enter_context(tc.tile_pool(name="scp", bufs=2))
        outp = ctx.enter_context(tc.tile_pool(name="outp", bufs=3))
        ps = ctx.enter_context(tc.tile_pool(name="ps", bufs=2, space="PSUM"))

        # +inf per-partition scalar for FP4_FIN's exponent mask (an inf
        # *immediate* is not JSON-serializable through walrus)
        inf_t = cst.tile([128, 1], dt.float32, tag="inf")
        nc.vector.memset(inf_t[:, :], float("inf"))
        bias_t = cst.tile([128, N_CORE], dt.bfloat16, tag="bias")
        nc.sync.dma_start(
            out=bias_t[:, :],
            in_=b_d[:].unsqueeze(0).broadcast_to([128, N_CORE]),
        )
        # resident transposed weights: WT[kp, nr, kc, n]
        WT = wtp.tile([128, NR, KC, 128], dt.bfloat16, tag="WT")

        # scale-chain scratch sections, packed in one tile: 8 x [128, KB]
        S_BM, S_SRAW, S_PE2, S_MG, S_TV, S_S, S_SH, S_RINV = range(8)

        def quant_tile(st):
            """Quantize+dequantize a pre-loaded [128, K] f32 stage tile:
            returns bf16 [128, K] tile.  Runs entirely on VectorE so the
            quant pipeline never waits on a PE-gated queue."""
            st3 = st[:, :].rearrange("p (b e) -> p b e", e=16)
            sc = scp.tile([128, 8 * KB], dt.float32, tag="sc", name="sc")

            def sec(i):
                return sc[:, i * KB:(i + 1) * KB]

            nc.vector.tensor_reduce(
                sec(S_BM), st3, axis=mybir.AxisListType.X, op=Alu.max,
                apply_absolute_value=True,
            )
            nc.vector.tensor_scalar(
                sec(S_SRAW), sec(S_BM), RCP6, float(2.0**-9), Alu.mult, Alu.max)
            nc.vector.tensor_scalar(
                sec(S_PE2).bitcast(dt.int32), sec(S_SRAW).bitcast(dt.int32),
                0x7F800000, None, Alu.bitwise_and)
            nc.vector.tensor_scalar_max(sec(S_PE2), sec(S_PE2), float(2.0**-6))
            # RNE of sraw onto the e4m3 grid (ulp pe2/8) via per-block magic add
            nc.vector.tensor_scalar_mul(sec(S_MG), sec(S_PE2), MAGIC)
            nc.vector.tensor_tensor(sec(S_TV), sec(S_SRAW), sec(S_MG), Alu.add)
            nc.vector.tensor_tensor(sec(S_S), sec(S_TV), sec(S_MG), Alu.subtract)
            nc.vector.tensor_scalar_mul(sec(S_SH), sec(S_S), 0.5)
            nc.vector.reciprocal_approx_fast(out=sec(S_RINV), in_=sec(S_S))

            qp = qpp.tile([128, K], dt.float32, tag="qpre", name="qp")
            qp3 = qp[:, :].rearrange("p (b e) -> p b e", e=16)
            nc.vector._custom_dve(
                fp4_pre, out=qp3, in0=st3,
                in1=sec(S_RINV).unsqueeze(2).broadcast_to([128, KB, 16]),
                s0=4.0, s1=CH1,
            )
            dq = dqp.tile([128, K], dt.bfloat16, tag="dq", name="dq")
            dq3 = dq[:, :].rearrange("p (b e) -> p b e", e=16)
            nc.vector._custom_dve(
                fp4_fin, out=dq3, in0=qp3,
                in1=sec(S_SH).unsqueeze(2).broadcast_to([128, KB, 16]),
                s0=inf_t[:, 0:1], s1=12.0,
            )
            return dq

        # --- wavefront schedule ---------------------------------------------
        # Quant tasks run in order [stripe0 w-tiles, x0, stripe1, x1,
        # stripe2, x2, x3, ...]; loads are emitted two tasks ahead so the
        # SP queue never starves VectorE.  Round r runs the matmul chains
        # on the anti-diagonal {(mt, nb): mt + nb == r} — stripe nb only
        # gates n-block nb, so TensorE starts after 4 of 12 w-tiles, and
        # 2 of 3 chains per round read older XT tiles (DVE jitter slack).
        # Queue roles (all in-order per engine): SP = loads + transposes
        # (producer side, never PE-gated); ACT = PSUM evac + out stores;
        # GpSimd = bias adds (both terminal, PE-gated is fine there).
        # x row-tiles are loaded and quantized in PAIRS (one 3MB DMA per two
        # tiles) and out rows stored in pairs — halving the DMA count keeps
        # the xbar-transition-serialized [load | transpose | store] chain
        # well under the per-round matmul budget.
        stage_tiles = {}

        def emit_wload(nr):
            st = stg.tile([128, 2, K], dt.float32, tag="stage",
                          name=f"st_w{nr}")
            nc.sync.dma_start(out=st[:, 0, :],
                              in_=w_d[nr * 128:(nr + 1) * 128, :])
            stage_tiles[("w", nr)] = st

        def emit_xload(mt0):
            st = stg.tile([128, 2, K], dt.float32, tag="stage",
                          name=f"st_x{mt0}")
            nc.sync.dma_start(
                out=st[:, :, :],
                in_=x_d[mt0 * 128:(mt0 + 2) * 128, :].rearrange(
                    "(t p) c -> p t c", t=2))
            stage_tiles[("x", mt0)] = st

        xt_tiles = {}
        ot_tiles = {}

        # quant-task order: all W first (TensorE can't run full-width
        # chains until every stripe exists, and interleaving x work only
        # delays the last stripe — W-first minimizes total PE idle), then
        # x pairs.  Loads are pumped from this list two tasks ahead (= the
        # 2 stage slots), each emitted right AFTER a transpose so the SP
        # queue never parks a load (whose slot frees later) ahead of a
        # transpose the VectorE pipeline needs (deadlock otherwise).
        order = [("w", nr) for nr in range(2 * TPB)]
        order.append(("x", 0))
        order += [("w", nr) for nr in range(2 * TPB, NR)]
        order += [("x", m0) for m0 in range(2, MT, 2)]
        load_i = [0]

        def pump_load():
            if load_i[0] < len(order):
                kind, idx = order[load_i[0]]
                (emit_wload if kind == "w" else emit_xload)(idx)
                load_i[0] += 1

        def quant_w(nr):
            st = stage_tiles.pop(("w", nr))
            dq = quant_tile(st[:, 0, :])
            nc.sync.dma_start_transpose(WT[:, nr, :, :], dq[:, :])
            pump_load()

        def quant_x(mt0):
            st = stage_tiles[("x", mt0)]
            for t in range(2):
                dq = quant_tile(st[:, t, :])
                xt = xtp.tile([128, KC, 128], dt.bfloat16, tag="XT",
                              name=f"xt{mt0 + t}")
                nc.sync.dma_start_transpose(xt[:, :, :], dq[:, :])
                xt_tiles[mt0 + t] = xt
                if t == 0:
                    pump_load()
            del stage_tiles[("x", mt0)]

        def emit_chain(mt, nb):
            pmm = ps.tile([128, 512], dt.float32, tag=f"mm{nb}",
                          name=f"pmm{mt}_{nb}")
            xt = xt_tiles[mt]
            for kc in range(KC):
                nc.tensor.matmul(
                    pmm[:, :], xt[:, kc, :],
                    WT[:, nb * TPB:(nb + 1) * TPB, kc, :],
                    start=(kc == 0), stop=(kc == KC - 1),
                )
            p0 = mt - (mt % 2)
            if nb == 0 and mt % 2 == 0:
                ot_tiles[p0] = outp.tile([128, 2, N_CORE], dt.bfloat16,
                                         tag="ot", name=f"ot{p0}")
            ot = ot_tiles[p0]
            sl = slice(nb * 512, (nb + 1) * 512)
            nc.scalar.copy(ot[:, mt % 2, sl], pmm[:, :])
            nc.gpsimd.tensor_tensor(
                ot[:, mt % 2, sl], ot[:, mt % 2, sl], bias_t[:, sl], Alu.add)
            if nb == NB - 1 and mt % 2 == 1:
                # SWDGE pair store: off the HWDGE completion lanes, and one
                # DMA per two m-tiles.
                nc.gpsimd.dma_start(
                    out=out_d[p0 * 128:(p0 + 2) * 128, :].rearrange(
                        "(t p) n -> p t n", t=2),
                    in_=ot[:, :, :])
                del ot_tiles[p0], xt_tiles[mt - 1], xt_tiles[mt]

        # prologue: two loads in flight, then quants pump the rest.
        # The first x-pair is quantized after stripes 0-1 so its n-block
        # 0/1 chains overlap the last stripe's quant on VectorE.
        pump_load()
        pump_load()
        for nr in range(2 * TPB):
            quant_w(nr)
        quant_x(0)
        for t in range(2):
            for nb in range(NB - 1):
                emit_chain(t, nb)
        for nr in range(2 * TPB, NR):
            quant_w(nr)
        for t in range(2):
            emit_chain(t, NB - 1)
        for m0 in range(2, MT, 2):
            quant_x(m0)
            for t in range(2):
                for nb in range(NB):
                    emit_chain(m0 + t, nb)

    if postprocess:
        _split_excess_waits(nc)
        # Raw Bass skips the ISA-byte encoding pass (Bacc.compile runs it);
        # without it custom-DVE/extended insts ship empty .instr -> walrus
        # "ISA wrong length".
        mybir.codegen_inst_isa_subclasses(nc)
    return nc


# ---------------------------------------------------------------------------
def _get_built():
    global _BUILT
    if _BUILT is None:
        _BUILT = build_nc()
    return _BUILT


def kernel(x, weight, bias):
    """Full-input entry point: x [2,4096,3072] f32, weight [12288,3072] f32,
    bias [12288] bf16 -> out [2,4096,12288] bf16."""
    from concourse.bass_utils import run_bass_kernel_spmd

    nc = _get_built()
    x2 = np.ascontiguousarray(np.asarray(x, dtype=f32).reshape(M, K))
    w = np.ascontiguousarray(np.asarray(weight, dtype=f32))
    b = np.asarray(bias)
    if b.dtype != bf16:
        if b.dtype.itemsize == 2 and b.dtype.kind in "Vu":
            b = b.view(bf16)
        else:
            b = b.astype(bf16)

    in_maps = []
    for c in range(NUM_CORES):
        in_maps.append({
            "x": x2,
            "w": w[c * N_CORE:(c + 1) * N_CORE],
            "bias": b[c * N_CORE:(c + 1) * N_CORE],
        })

    res = run_bass_kernel_spmd(nc, in_maps, list(range(NUM_CORES)))
    out = np.empty((M, N), dtype=bf16)
    for c in range(NUM_CORES):
        out[:, c * N_CORE:(c + 1) * N_CORE] = (
            np.asarray(res.results[c]["out"]).astype(bf16, copy=False)
        )
    return out.reshape(B, T, N)


# revision 28
# speedup vs baseline: 1.0223x; 1.0223x over previous
"""NVFP4 block-quantized linear layer (x @ w.T + bias) on 8 Trainium2 cores.

Reference semantics (reference.py): both activations and weights are
block-quantized along K (blocks of 16) to fp4-e2m1 with e4m3 scales
(scale = absmax/6, round-to-nearest), dequantized, then matmul with fp32
accumulation, cast to bf16, plus bf16 bias.

Device strategy: pure column-parallel over 8 cores (N 12288 -> 1536/core),
x replicated.  Per core:
  - weights quantized once (12 row-tiles), xbar-DMA-transposed into a
    resident SBUF tensor WT [128k, 12nr, 24kc, 128n] bf16 (74KB/partition).
  - activations quantized per 128-row tile on VectorE (blockwise absmax,
    e4m3 RNE scale via per-block magic-add, fp4 round via two custom DVE
    ops), xbar-DMA-transposed to XT [128k, 24kc, 128m].
  - TensorE: for each m-tile, 24kc x 3nb bf16 matmuls accumulate in 6
    PSUM banks (double-buffered across m-tiles); ScalarE evacuates to
    bf16, GpSimd adds bias (bf16, matching the reference's double
    rounding), one 384KB DMA stores each m-tile row block.
All DMA is HWDGE; transposes ride the DMA xbar so TensorE runs nothing
but the 4608 matmuls (216ns each at N=512 -> ~1ms roofline).
"""

import numpy as np
import ml_dtypes

f32 = np.float32
bf16 = ml_dtypes.bfloat16

# ---------------------------------------------------------------------------
# problem geometry (hardcoded; harness calls kernel() with these full shapes)
B, T, K = 2, 4096, 3072
N = 12288
M = B * T                      # 8192
GRID_M, GRID_N = 1, 8          # column-parallel
M_CORE = M                     # 8192
N_CORE = N // GRID_N           # 1536
NUM_CORES = GRID_M * GRID_N

KC = K // 128                  # 24 k-chunks
KB = K // 16                   # 192 scale blocks per row
NR = N_CORE // 128             # 12 weight row-tiles
MT = M_CORE // 128             # 64 x row-tiles
NB = N_CORE // 512             # 3 psum n-blocks
TPB = 4                        # w row-tiles per 512-wide n-block

CH1 = float(1.5 * 2**22)
RCP6 = float(f32(1.0) / f32(6.0))
MAGIC = float(1.5 * 2**20)     # pe2*MAGIC has ulp = pe2/8 (e4m3 mantissa grid)

_BUILT = None


# ---------------------------------------------------------------------------
def _register_custom_ops():
    """Register the two fp4-rounding custom DVE ops (idempotent)."""
    import concourse.dve_ops as dve_ops
    from concourse.dve_ops import DveOp, OPS, _SUB_OPCODE_FOR_NAME, _CUSTOM_DVE_ROW_BASE
    from concourse.dve_spec import (
        Spec, Src0, Src1, C0, C1, Zero, One, AluOp, Bin,
        maxx, minn, select, lower, _has_src1,
    )
    from concourse.dve_uop import DveOpSpec

    def _norm2(in0, in1):
        in0 = np.asarray(in0)
        in1 = np.asarray(in1)
        if in1.size != in0.size:
            in1 = np.broadcast_to(in1, in0.shape)
        return in0, np.ascontiguousarray(in1).reshape(in0.shape)

    def _ref_fp4_pre(in0, in1, s0, s1, imm2=None):
        in0, in1 = _norm2(in0, in1)
        m = (in0.astype(f32) * in1.astype(f32)).astype(f32)
        s2 = (m * m).astype(f32)
        ch = np.where(
            s2 < f32(4.0), f32(CH1),
            ((f32(1.0) + (s2 >= f32(16.0)).astype(f32)) * f32(1.5 * 2**23)).astype(f32),
        ).astype(f32)
        return (m + ch).astype(f32)

    def _ref_fp4_fin(in0, in1, s0, s1, imm2=None):
        in0, in1 = _norm2(in0, in1)
        qpre = np.ascontiguousarray(in0.astype(f32))
        pe = (qpre.view(np.uint32) & np.uint32(0x7F800000)).view(f32)
        d1 = (qpre - pe).astype(f32)
        q2 = ((d1 + d1).astype(f32) - pe).astype(f32)
        qc = np.maximum(np.minimum(q2, f32(12.0)), f32(-12.0))
        return (qc * in1.astype(f32)).astype(f32)

    def build_pre():
        SIXTEEN = C0 * C0
        Ch2x = C1 + C1
        m = Src0 * Src1
        s2 = m * m
        c2 = s2 >= SIXTEEN
        inner = (c2 + One) * Ch2x
        c1 = s2 < C0
        outer = select(c1, C1, inner)
        return Spec(body=m + outer, reference=_ref_fp4_pre)

    def build_fin():
        pe = Bin(AluOp.BITWISE_AND, Src0, C0)
        d1 = Src0 - pe
        q2 = (d1 + d1) - pe
        qc = maxx(minn(q2, C1), Zero - C1)
        return Spec(body=qc * Src1, reference=_ref_fp4_fin)

    def register(name, spec):
        if name in _SUB_OPCODE_FOR_NAME:
            for op in OPS:
                if op.name == name:
                    return op
            raise RuntimeError(name)
        row = _CUSTOM_DVE_ROW_BASE + len(OPS)
        assert row < 0x20
        shas = {}
        for ver in ("v3", "v4"):
            try:
                uops = lower(spec, ver=ver)
            except Exception:
                continue
            shas[ver] = DveOpSpec(
                name=name, opcode=row, uops=uops, rd1_en=_has_src1(spec)
            ).sha(ver)
        op = DveOp(name, spec, subdim=False, uops_sha=shas)
        OPS.append(op)
        _SUB_OPCODE_FOR_NAME[name] = row
        dve_ops.CUSTOM_DVE_SPECS[name] = spec
        return op

    return register("FP4_PRE_ANT", build_pre()), register("FP4_FIN_ANT", build_fin())


# ---------------------------------------------------------------------------
def _patch_tile_drain():
    """The TileContext tail drain attaches one sem-wait per live logical
    processor to a single SP Drain instruction; this walrus build caps sync
    waits per instruction at 2 ("Too many sync wait commands").  Split the
    overflow waits onto preceding single-wait SP nops (sound: all waits still
    complete before the post-drain all-engine barrier / sem reset)."""
    from concourse import tile as tile_mod
    import concourse.mybir as mybir
    from concourse.vector_clock import ScopedClock

    if getattr(tile_mod.TileContext, "_ant_drain_patched", False):
        return

    def _drain_and_barrier(self, tick_clock, wait_clock):
        nc = self.nc
        probe = nc.sync.nop()
        wait_clock.add_sem_waits(
            probe.ins, ScopedClock({None: tick_clock.global_clock})
        )
        si = probe.ins.sync_info
        waits = list(si.on_wait) if si is not None and si.on_wait else []
        if len(waits) > 1:
            probe.ins.sync_info = mybir.SyncInfo(
                on_wait=waits[:1],
                on_update=list(si.on_update) if si.on_update else [],
            )
            for w in waits[1:]:
                extra = nc.sync.nop()
                extra.ins.sync_info = mybir.SyncInfo(on_wait=[w], on_update=[])
        nc.sync.drain()

        nc.all_engine_barrier()
        assert self.sems is not None
        popped = nc._tile_sem_poison_stack.pop()
        assert popped is self._sem_poison
        nc.clear_and_free_semaphores(list(self.sems.allocated().values()))
        nc.all_engine_barrier()

    tile_mod.TileContext._drain_and_barrier = _drain_and_barrier
    tile_mod.TileContext._ant_drain_patched = True


def _split_excess_waits(nc, max_waits=1):
    """This walrus build rejects instructions carrying more than `max_waits`
    sem waits ("Too many sync wait commands").  Move overflow waits onto
    same-engine NoOp instructions inserted immediately before the offender —
    per-engine program order makes this semantically identical."""
    import concourse.mybir as mybir

    ctr = [0]
    for f in nc.m.functions:
        for blk in f.blocks:
            il = blk.instructions
            out = []
            changed = False
            for ins in il:
                si = ins.sync_info
                waits = list(si.on_wait) if si is not None and si.on_wait else []
                if len(waits) > max_waits:
                    changed = True
                    extra = waits[:-max_waits]
                    for i0 in range(0, len(extra), max_waits):
                        nop = mybir.InstNoOp(
                            name=f"I-waitsplit-{ctr[0]}", ins=[], outs=[])
                        ctr[0] += 1
                        nop.engine = ins.engine
                        nop.sync_info = mybir.SyncInfo(
                            on_wait=extra[i0:i0 + max_waits], on_update=[])
                        out.append(nop)
                    ins.sync_info = mybir.SyncInfo(
                        on_wait=waits[-max_waits:],
                        on_update=list(si.on_update) if si.on_update else [],
                    )
                out.append(ins)
            if changed:
                blk.instructions = out


def build_nc(num_cores=NUM_CORES, debug=False, postprocess=True):
    """Build the per-core Bass program (SPMD: same program on every core)."""
    import concourse.bass as bass
    import concourse.mybir as mybir
    from concourse import tile
    from contextlib import ExitStack

    fp4_pre, fp4_fin = _register_custom_ops()
    _patch_tile_drain()

    nc = bass.Bass("TRN2", target_bir_lowering=False, debug=debug,
                   num_devices=num_cores)
    dt = mybir.dt
    Alu = mybir.AluOpType

    x_d = nc.dram_tensor("x", [M_CORE, K], dt.float32, kind="ExternalInput")
    w_d = nc.dram_tensor("w", [N_CORE, K], dt.float32, kind="ExternalInput")
    b_d = nc.dram_tensor("bias", [N_CORE], dt.bfloat16, kind="ExternalInput")
    out_d = nc.dram_tensor("out", [M_CORE, N_CORE], dt.bfloat16,
                           kind="ExternalOutput")

    with tile.TileContext(nc) as tc, ExitStack() as ctx:
        cst = ctx.enter_context(tc.tile_pool(name="cst", bufs=1))
        wtp = ctx.enter_context(tc.tile_pool(name="wtp", bufs=1))
        xtp = ctx.enter_context(tc.tile_pool(name="xtp", bufs=4))
        stg = ctx.enter_context(tc.tile_pool(name="stg", bufs=2))
        qpp = ctx.enter_context(tc.tile_pool(name="qpp", bufs=1))
        dqp = ctx.enter_context(tc.tile_pool(name="dqp", bufs=2))
        scp = ctx.enter_context(tc.tile_pool(name="scp", bufs=2))
        outp = ctx.enter_context(tc.tile_pool(name="outp", bufs=3))
        ps = ctx.enter_context(tc.tile_pool(name="ps", bufs=2, space="PSUM"))

        # +inf per-partition scalar for FP4_FIN's exponent mask (an inf
        # *immediate* is not JSON-serializable through walrus)
        inf_t = cst.tile([128, 1], dt.float32, tag="inf")
        nc.vector.memset(inf_t[:, :], float("inf"))
        bias_t = cst.tile([128, N_CORE], dt.bfloat16, tag="bias")
        nc.sync.dma_start(
            out=bias_t[:, :],
            in_=b_d[:].unsqueeze(0).broadcast_to([128, N_CORE]),
        )
        # resident transposed weights: WT[kp, nr, kc, n]
        WT = wtp.tile([128, NR, KC, 128], dt.bfloat16, tag="WT")

        # scale-chain scratch sections, packed in one tile: 8 x [128, KB]
        S_BM, S_SRAW, S_PE2, S_MG, S_TV, S_S, S_SH, S_RINV = range(8)

        def quant_tile(st):
            """Quantize+dequantize a pre-loaded [128, K] f32 stage tile:
            returns bf16 [128, K] tile.  Runs entirely on VectorE so the
            quant pipeline never waits on a PE-gated queue."""
            st3 = st[:, :].rearrange("p (b e) -> p b e", e=16)
            sc = scp.tile([128, 8 * KB], dt.float32, tag="sc", name="sc")

            def sec(i):
                return sc[:, i * KB:(i + 1) * KB]

            nc.vector.tensor_reduce(
                sec(S_BM), st3, axis=mybir.AxisListType.X, op=Alu.max,
                apply_absolute_value=True,
            )
            nc.vector.tensor_scalar(
                sec(S_SRAW), sec(S_BM), RCP6, float(2.0**-9), Alu.mult, Alu.max)
            nc.vector.tensor_scalar(
                sec(S_PE2).bitcast(dt.int32), sec(S_SRAW).bitcast(dt.int32),
                0x7F800000, None, Alu.bitwise_and)
            nc.vector.tensor_scalar_max(sec(S_PE2), sec(S_PE2), float(2.0**-6))
            # RNE of sraw onto the e4m3 grid (ulp pe2/8) via per-block magic add
            nc.vector.tensor_scalar_mul(sec(S_MG), sec(S_PE2), MAGIC)
            nc.vector.tensor_tensor(sec(S_TV), sec(S_SRAW), sec(S_MG), Alu.add)
            nc.vector.tensor_tensor(sec(S_S), sec(S_TV), sec(S_MG), Alu.subtract)
            nc.vector.tensor_scalar_mul(sec(S_SH), sec(S_S), 0.5)
            nc.vector.reciprocal_approx_fast(out=sec(S_RINV), in_=sec(S_S))

            qp = qpp.tile([128, K], dt.float32, tag="qpre", name="qp")
            qp3 = qp[:, :].rearrange("p (b e) -> p b e", e=16)
            nc.vector._custom_dve(
                fp4_pre, out=qp3, in0=st3,
                in1=sec(S_RINV).unsqueeze(2).broadcast_to([128, KB, 16]),
                s0=4.0, s1=CH1,
            )
            dq = dqp.tile([128, K], dt.bfloat16, tag="dq", name="dq")
            dq3 = dq[:, :].rearrange("p (b e) -> p b e", e=16)
            nc.vector._custom_dve(
                fp4_fin, out=dq3, in0=qp3,
                in1=sec(S_SH).unsqueeze(2).broadcast_to([128, KB, 16]),
                s0=inf_t[:, 0:1], s1=12.0,
            )
            return dq

        # --- wavefront schedule ---------------------------------------------
        # Quant tasks run in order [stripe0 w-tiles, x0, stripe1, x1,
        # stripe2, x2, x3, ...]; loads are emitted two tasks ahead so the
        # SP queue never starves VectorE.  Round r runs the matmul chains
        # on the anti-diagonal {(mt, nb): mt + nb == r} — stripe nb only
        # gates n-block nb, so TensorE starts after 4 of 12 w-tiles, and
        # 2 of 3 chains per round read older XT tiles (DVE jitter slack).
        # Queue roles (all in-order per engine): SP = loads + transposes
        # (producer side, never PE-gated); ACT = PSUM evac + out stores;
        # GpSimd = bias adds (both terminal, PE-gated is fine there).
        # x row-tiles are loaded and quantized in PAIRS (one 3MB DMA per two
        # tiles) and out rows stored in pairs — halving the DMA count keeps
        # the xbar-transition-serialized [load | transpose | store] chain
        # well under the per-round matmul budget.
        stage_tiles = {}

        def emit_pload(t):
            """Load a PAIR of 128-row tiles (w or x) in one 3MB DMA."""
            kind, idx = t
            src = w_d if kind == "w" else x_d
            st = stg.tile([128, 2, K], dt.float32, tag="stage",
                          name=f"st_{kind}{idx}")
            nc.sync.dma_start(
                out=st[:, :, :],
                in_=src[idx * 128:(idx + 2) * 128, :].rearrange(
                    "(t p) c -> p t c", t=2))
            stage_tiles[t] = st

        xt_tiles = {}
        ot_tiles = {}

        # quant-task order: all W first (TensorE can't run full-width
        # chains until every stripe exists, and interleaving x work only
        # delays the last stripe — W-first minimizes total PE idle), then
        # x pairs.  Pair loads are pumped from this list two pairs ahead
        # (= the 2 stage slots), each emitted right AFTER a transpose so
        # the SP queue never parks a load (whose slot frees later) ahead
        # of a transpose the VectorE pipeline needs (deadlock otherwise).
        order = [("w", nr) for nr in range(0, NR, 2)]
        order += [("x", m0) for m0 in range(0, MT, 2)]
        load_i = [0]

        def pump_load():
            if load_i[0] < len(order):
                emit_pload(order[load_i[0]])
                load_i[0] += 1

        def quant_half(kind, pair, t):
            st = stage_tiles[(kind, pair)]
            dq = quant_tile(st[:, t, :])
            idx = pair + t
            if kind == "w":
                nc.sync.dma_start_transpose(WT[:, idx, :, :], dq[:, :])
            else:
                xt = xtp.tile([128, KC, 128], dt.bfloat16, tag="XT",
                              name=f"xt{idx}")
                nc.sync.dma_start_transpose(xt[:, :, :], dq[:, :])
                xt_tiles[idx] = xt
            if t == 0:
                pump_load()
            else:
                del stage_tiles[(kind, pair)]

        def emit_chain(mt, nb):
            pmm = ps.tile([128, 512], dt.float32, tag=f"mm{nb}",
                          name=f"pmm{mt}_{nb}")
            xt = xt_tiles[mt]
            for kc in range(KC):
                nc.tensor.matmul(
                    pmm[:, :], xt[:, kc, :],
                    WT[:, nb * TPB:(nb + 1) * TPB, kc, :],
                    start=(kc == 0), stop=(kc == KC - 1),
                )
            p0 = mt - (mt % 2)
            if nb == 0 and mt % 2 == 0:
                ot_tiles[p0] = outp.tile([128, 2, N_CORE], dt.bfloat16,
                                         tag="ot", name=f"ot{p0}")
            ot = ot_tiles[p0]
            sl = slice(nb * 512, (nb + 1) * 512)
            nc.scalar.copy(ot[:, mt % 2, sl], pmm[:, :])
            nc.gpsimd.tensor_tensor(
                ot[:, mt % 2, sl], ot[:, mt % 2, sl], bias_t[:, sl], Alu.add)
            if nb == NB - 1 and mt % 2 == 1:
                # SWDGE pair store: off the HWDGE completion lanes, and one
                # DMA per two m-tiles.
                nc.gpsimd.dma_start(
                    out=out_d[p0 * 128:(p0 + 2) * 128, :].rearrange(
                        "(t p) n -> p t n", t=2),
                    in_=ot[:, :, :])
                del ot_tiles[p0], xt_tiles[mt - 1], xt_tiles[mt]

        # prologue: two pair-loads in flight, then quants pump the rest.
        # Each x tile's chains are emitted right after its own transpose
        # (not after the whole pair) so TensorE starts as early as possible
        # and tracks the quant pipeline at tile granularity.
        pump_load()
        pump_load()
        for p in range(0, NR, 2):
            quant_half("w", p, 0)
            quant_half("w", p, 1)
        for m0 in range(0, MT, 2):
            for t in range(2):
                quant_half("x", m0, t)
                for nb in range(NB):
                    emit_chain(m0 + t, nb)

    if postprocess:
        _split_excess_waits(nc)
        # Raw Bass skips the ISA-byte encoding pass (Bacc.compile runs it);
        # without it custom-DVE/extended insts ship empty .instr -> walrus
        # "ISA wrong length".
        mybir.codegen_inst_isa_subclasses(nc)
    return nc


# ---------------------------------------------------------------------------
def _get_built():
    global _BUILT
    if _BUILT is None:
        _BUILT = build_nc()
    return _BUILT


def kernel(x, weight, bias):
    """Full-input entry point: x [2,4096,3072] f32, weight [12288,3072] f32,
    bias [12288] bf16 -> out [2,4096,12288] bf16."""
    from concourse.bass_utils import run_bass_kernel_spmd

    nc = _get_built()
    x2 = np.ascontiguousarray(np.asarray(x, dtype=f32).reshape(M, K))
    w = np.ascontiguousarray(np.asarray(weight, dtype=f32))
    b = np.asarray(bias)
    if b.dtype != bf16:
        if b.dtype.itemsize == 2 and b.dtype.kind in "Vu":
            b = b.view(bf16)
        else:
            b = b.astype(bf16)

    in_maps = []
    for c in range(NUM_CORES):
        in_maps.append({
            "x": x2,
            "w": w[c * N_CORE:(c + 1) * N_CORE],
            "bias": b[c * N_CORE:(c + 1) * N_CORE],
        })

    res = run_bass_kernel_spmd(nc, in_maps, list(range(NUM_CORES)))
    out = np.empty((M, N), dtype=bf16)
    for c in range(NUM_CORES):
        out[:, c * N_CORE:(c + 1) * N_CORE] = (
            np.asarray(res.results[c]["out"]).astype(bf16, copy=False)
        )
    return out.reshape(B, T, N)
